# revision 4
# baseline (speedup 1.0000x reference)
"""Causal self-attention (B=4, T=2048, C=2048, H=16, rope) on 8 trn2 cores.

Sharding: tensor-parallel over heads — 2 heads per core. Each core computes
q/k/v projections for its head slice from the full x, runs causal attention,
and produces a partial output projection y_c = attn_c @ wo[:, slice].T.
The host sums the 8 partial y tensors (row-parallel linear unshard).

Kernel layout (per core, all "T" suffixes = transposed so the contraction
dim sits on SBUF partitions):
  qT/kT [d=128, t]  <- wT (stationary) x xT (moving) matmuls + rope on DVE
  v     [t=128, d]  <- PE transpose of vT tiles
  S^T   [j, i]      <- kT-tile (stationary) x qT-block (moving)
  P^T = exp(S^T * scale) with additive causal mask pre-exp
  attn^T [d, i]     <- v-tile (stationary) x P^T (moving), PSUM-accumulated
  rowsum broadcast  <- ones[128,128] (stationary) x rs-partial (moving)
  y[n, j]           <- attnT-tile (stationary) x woT (moving)
"""

import numpy as np

import concourse.bass as bass
import concourse.mybir as mybir
import concourse.tile as tile
from concourse.vector_clock import ScopedClock
from concourse.bass_utils import run_bass_kernel_spmd

# ---------------------------------------------------------------- tile patch
# The pinned walrus codegen accepts at most ONE sync-wait per hardware
# instruction; Tile attaches several. Split extras onto same-engine NOPs.

_MAX_WAITS = 1
_orig_add_instruction = tile.TileContext._add_instruction


def _split_add_instruction(self, inst):
    si = getattr(inst, "sync_info", None)
    if si is not None and len(si.on_wait) > _MAX_WAITS:
        waits = list(si.on_wait)
        extras, keep = waits[:-_MAX_WAITS], waits[-_MAX_WAITS:]
        inst.sync_info = mybir.SyncInfo(on_wait=keep, on_update=list(si.on_update))
        for i in range(0, len(extras), _MAX_WAITS):
            nop = mybir.InstNoOp(
                name=f"{inst.name}-ws{i}",
                sync_info=mybir.SyncInfo(on_wait=extras[i : i + _MAX_WAITS], on_update=[]),
                engine=inst.engine,
                bass_nofuse=True,
            )
            _orig_add_instruction(self, nop)
    _orig_add_instruction(self, inst)


def _patched_drain_and_barrier(self, tick_clock, wait_clock):
    nc = self.nc
    drain_inst = nc.sync.drain()
    wait_clock.add_sem_waits(drain_inst.ins, ScopedClock({None: tick_clock.global_clock}))
    si = drain_inst.ins.sync_info
    if si is not None and len(si.on_wait) > 1:
        waits = list(si.on_wait)
        drain_inst.ins.sync_info = mybir.SyncInfo(on_wait=waits[:1], on_update=list(si.on_update))
        for w in waits[1:]:
            extra = nc.sync.drain()
            extra.ins.sync_info = mybir.SyncInfo(on_wait=[w], on_update=[])
    nc.all_engine_barrier()
    assert self.sems is not None
    popped = nc._tile_sem_poison_stack.pop()
    assert popped is self._sem_poison
    nc.clear_and_free_semaphores(list(self.sems.allocated().values()))
    nc.all_engine_barrier()


tile.TileContext._add_instruction = _split_add_instruction
tile.TileContext._drain_and_barrier = _patched_drain_and_barrier

# ---------------------------------------------------------------- constants

B, T, C, H, D = 4, 2048, 2048, 16, 128
N_CORES = 8
HPC = H // N_CORES        # heads per core = 2
M = HPC * D               # per-core projection width = 256
BT = B * T
KT = C // 128             # 16 k-subtiles
SCALE = 1.0 / float(np.sqrt(D))
NEG = -30000.0            # pre-scale additive mask value; exp(scale*NEG+x) == 0

F32 = mybir.dt.float32
F32R = mybir.dt.float32r

# matmul dtype for the heavy stages (flip to F32 for a full-precision run)
DT_MM = F32R
ALU = mybir.AluOpType
AF = mybir.ActivationFunctionType


def build_kernel(dt_mm=DT_MM):
    nc = bass.Bass("TRN2", target_bir_lowering=False, debug=False)

    xT = nc.dram_tensor("xT", [C, BT], dt_mm, kind="ExternalInput").ap()
    wqT = nc.dram_tensor("wqT", [C, M], dt_mm, kind="ExternalInput").ap()
    wkT = nc.dram_tensor("wkT", [C, M], dt_mm, kind="ExternalInput").ap()
    wvT = nc.dram_tensor("wvT", [C, M], dt_mm, kind="ExternalInput").ap()
    woT = nc.dram_tensor("woT", [M, C], dt_mm, kind="ExternalInput").ap()
    cosT = nc.dram_tensor("cosT", [D, T], F32, kind="ExternalInput").ap()
    sinT = nc.dram_tensor("sinT", [D, T], F32, kind="ExternalInput").ap()
    maskA = nc.dram_tensor("maskA", [128, 896], F32, kind="ExternalInput").ap()
    ones = nc.dram_tensor("ones", [128, 128], F32, kind="ExternalInput").ap()
    ident = nc.dram_tensor("ident", [128, 128], F32, kind="ExternalInput").ap()
    y = nc.dram_tensor("y", [BT, C], F32, kind="ExternalOutput").ap()

    xT3 = xT.rearrange("(ko p) n -> p ko n", p=128)

    with tile.TileContext(nc) as tc:
        with (
            tc.tile_pool(name="const", bufs=1) as constp,
            tc.tile_pool(name="xpool", bufs=2) as xpool,
            tc.tile_pool(name="qpool", bufs=2) as qpool,
            tc.tile_pool(name="kvpool", bufs=1) as kvpool,
            tc.tile_pool(name="attnpool", bufs=2) as attnpool,
            tc.tile_pool(name="vstg", bufs=2) as vstg,
            tc.tile_pool(name="ptpool", bufs=3) as ptpool,
            tc.tile_pool(name="rspool", bufs=2) as rspool,
            tc.tile_pool(name="tmp", bufs=4) as tmpp,
            tc.tile_pool(name="ystg", bufs=3) as ystg,
            tc.tile_pool(name="ps_main", bufs=3, space="PSUM") as ps_main,
            tc.tile_pool(name="ps_misc", bufs=3, space="PSUM") as ps_misc,
            tc.tile_pool(name="ps_av", bufs=2, space="PSUM") as ps_av,
        ):
            # ---- resident constants
            wq_sb = constp.tile([128, KT, M], dt_mm, tag="wq")
            wk_sb = constp.tile([128, KT, M], dt_mm, tag="wk")
            wv_sb = constp.tile([128, KT, M], dt_mm, tag="wv")
            nc.sync.dma_start(wq_sb[:], wqT.rearrange("(ko p) m -> p ko m", p=128))
            nc.sync.dma_start(wk_sb[:], wkT.rearrange("(ko p) m -> p ko m", p=128))
            nc.sync.dma_start(wv_sb[:], wvT.rearrange("(ko p) m -> p ko m", p=128))
            wo_sb = constp.tile([128, HPC, C], dt_mm, tag="wo")
            nc.sync.dma_start(wo_sb[:], woT.rearrange("(mh p) j -> p mh j", p=128))
            cos_sb = constp.tile([D, T], F32, tag="cos")
            sin_sb = constp.tile([D, T], F32, tag="sin")
            nc.sync.dma_start(cos_sb[:], cosT[:])
            nc.sync.dma_start(sin_sb[:], sinT[:])
            mask_sb = constp.tile([128, 896], F32, tag="mask")
            nc.sync.dma_start(mask_sb[:], maskA[:])
            ones_sb = constp.tile([128, 128], F32, tag="ones")
            nc.sync.dma_start(ones_sb[:], ones[:])
            id_sb = constp.tile([128, 128], F32, tag="ident")
            nc.sync.dma_start(id_sb[:], ident[:])

            wqkv = [wq_sb, wq_sb, wk_sb, wk_sb, wv_sb, wv_sb]

            for b in range(B):
                # k/v for the whole sequence of this batch accumulate here
                k_sb = kvpool.tile([D, HPC, T], dt_mm, tag="k")
                v_sb = kvpool.tile([128, HPC, T // 128, D], dt_mm, tag="v")

                for a in range(4):  # 512-token block (QKV -> attn -> proj)
                    q_sb = qpool.tile([D, HPC, 512], dt_mm, tag="q")
                    attn_sb = attnpool.tile([D, HPC, 512], dt_mm, tag="attn")

                    # ---------------- phase A: qkv + rope for tokens [a*512, a*512+512)
                    for nb in range(2):
                        t0 = a * 512 + nb * 256
                        x_t = xpool.tile([128, KT, 256], dt_mm, tag="x")
                        nc.sync.dma_start(x_t[:], xT3[:, :, b * T + t0 : b * T + t0 + 256])
                        for m in range(6):
                            h = m % 2
                            ps_full = ps_main.tile([128, 512], F32, tag="mm", name="mm")
                            ps = ps_full[:, :256]
                            w_sb = wqkv[m]
                            for kt in range(KT):
                                nc.tensor.matmul(
                                    ps,
                                    w_sb[:, kt, h * D : (h + 1) * D],
                                    x_t[:, kt, :],
                                    start=(kt == 0),
                                    stop=(kt == KT - 1),
                                )
                            if m < 4:  # q/k: rope
                                rot_full = tmpp.tile([128, 512], F32, tag="tmp", name="rot")
                                rot = rot_full[:, :256]
                                t1_full = tmpp.tile([128, 512], F32, tag="tmp", name="t1")
                                t1 = t1_full[:, :256]
                                nc.vector.tensor_scalar_mul(rot[0:64, :], ps[64:128, :], -1.0)
                                nc.vector.tensor_copy(rot[64:128, :], ps[0:64, :])
                                nc.vector.tensor_tensor(t1, ps, cos_sb[:, t0 : t0 + 256], ALU.mult)
                                nc.vector.tensor_tensor(rot, rot, sin_sb[:, t0 : t0 + 256], ALU.mult)
                                dst = q_sb if m < 2 else k_sb
                                col = nb * 256 if m < 2 else t0
                                nc.vector.tensor_tensor(
                                    dst[:, h, col : col + 256], t1, rot, ALU.add
                                )
                            else:  # v: transpose to [t, d]
                                vt = vstg.tile([128, 256], F32, tag="vt")
                                nc.vector.tensor_copy(vt[:], ps)
                                for u in range(2):
                                    pst_full = ps_misc.tile([128, 512], F32, tag="misc", name="pst")
                                    pst = pst_full[:, :128]
                                    nc.tensor.transpose(pst, vt[:, u * 128 : (u + 1) * 128], id_sb[:])
                                    jt = (t0 + u * 128) // 128
                                    nc.vector.tensor_copy(v_sb[:, h, jt, :], pst)

                    # ---------------- phase B: attention for i-block a, both heads
                    njt = 4 * a + 4
                    for h in range(HPC):
                        av = ps_av.tile([128, 512], F32, tag="av")
                        rs = rspool.tile([128, 512], F32, tag="rs")
                        for jt in range(njt):
                            sp = ps_misc.tile([128, 512], F32, tag="misc")
                            nc.tensor.matmul(
                                sp,
                                k_sb[:, h, jt * 128 : (jt + 1) * 128],
                                q_sb[:, h, :],
                                start=True,
                                stop=True,
                            )
                            if jt >= 4 * a:  # diagonal block: additive causal mask
                                o = jt * 128 - a * 512
                                nc.vector.tensor_tensor(
                                    sp, sp, mask_sb[:, 384 - o : 896 - o], ALU.add
                                )
                            pt = ptpool.tile([128, 512], dt_mm, tag="pt")
                            nc.scalar.activation(pt[:], sp, AF.Exp, scale=SCALE)
                            ptf = pt[:].bitcast(F32)
                            if jt == 0:
                                nc.vector.tensor_copy(rs[:], ptf)
                            else:
                                nc.vector.tensor_tensor(rs[:], rs[:], ptf, ALU.add)
                            nc.tensor.matmul(
                                av,
                                v_sb[:, h, jt, :],
                                pt[:],
                                start=(jt == 0),
                                stop=(jt == njt - 1),
                            )
                        rsb = ps_misc.tile([128, 512], F32, tag="misc")
                        nc.tensor.matmul(rsb, ones_sb[:], rs[:], start=True, stop=True)
                        rec = tmpp.tile([128, 512], F32, tag="tmp")
                        nc.vector.reciprocal(rec[:], rsb)
                        nc.vector.tensor_tensor(attn_sb[:, h, :], av, rec[:], ALU.mult)

                    # ---------------- phase C: output projection for this block
                    for nt in range(4):
                        for jb in range(4):
                            yp = ps_main.tile([128, 512], F32, tag="mm")
                            for mh in range(HPC):
                                nc.tensor.matmul(
                                    yp,
                                    attn_sb[:, mh, nt * 128 : (nt + 1) * 128],
                                    wo_sb[:, mh, jb * 512 : (jb + 1) * 512],
                                    start=(mh == 0),
                                    stop=(mh == HPC - 1),
                                )
                            yt = ystg.tile([128, 512], F32, tag="y")
                            nc.any.tensor_copy(yt[:], yp)
                            r0 = b * T + a * 512 + nt * 128
                            nc.sync.dma_start(
                                y[r0 : r0 + 128, jb * 512 : (jb + 1) * 512], yt[:]
                            )
    return nc


_NC_CACHE = {}


def _get_nc(dt_mm):
    key = str(dt_mm)
    if key not in _NC_CACHE:
        _NC_CACHE[key] = build_kernel(dt_mm)
    return _NC_CACHE[key]


def make_inputs(x, freqs_cos, freqs_sin, wq, wk, wv, wo):
    """Host-side shard prep: returns in_maps for the 8 cores."""
    x = np.asarray(x, dtype=np.float32)
    xT = np.ascontiguousarray(x.reshape(BT, C).T)
    cosT = np.ascontiguousarray(np.asarray(freqs_cos, np.float32).T)
    sinT = np.ascontiguousarray(np.asarray(freqs_sin, np.float32).T)
    p = np.arange(128)[:, None]
    g = np.arange(896)[None, :]
    # additive pre-scale mask: 0 where j<=i (valid), NEG where masked
    maskA = np.where(p <= g - 384, 0.0, NEG).astype(np.float32)
    ones = np.ones((128, 128), np.float32)
    ident = np.eye(128, dtype=np.float32)
    in_maps = []
    for c in range(N_CORES):
        sl = slice(c * M, (c + 1) * M)
        in_maps.append(
            {
                "xT": xT,
                "wqT": np.ascontiguousarray(np.asarray(wq, np.float32)[sl, :].T),
                "wkT": np.ascontiguousarray(np.asarray(wk, np.float32)[sl, :].T),
                "wvT": np.ascontiguousarray(np.asarray(wv, np.float32)[sl, :].T),
                "woT": np.ascontiguousarray(np.asarray(wo, np.float32)[:, sl].T),
                "cosT": cosT,
                "sinT": sinT,
                "maskA": maskA,
                "ones": ones,
                "ident": ident,
            }
        )
    return in_maps


def kernel(x, freqs_cos, freqs_sin, wq, wk, wv, wo):
    nc = _get_nc(DT_MM)
    in_maps = make_inputs(x, freqs_cos, freqs_sin, wq, wk, wv, wo)
    res = run_bass_kernel_spmd(nc, in_maps, list(range(N_CORES)))
    out = np.zeros((BT, C), np.float64)
    for r in res.results:
        out += r["y"].astype(np.float64)
    return out.astype(np.float32).reshape(B, T, C)
